# revision 1
# baseline (speedup 1.0000x reference)
"""Grouped categorical log-softmax (segment logsumexp) on 8 Trainium2 cores.

Strategy: the index is sorted, so each segment is a contiguous run. On the host
we bucket segments by length (exact lengths 2..24, coarser canonical lengths for
the rare tail, padding inside a slot with -80 so exp() contributes nothing to
fp32 sums), shard every bucket evenly across the 8 cores, and lay each core's
data out as a dense [128, W_total] matrix where every bucket occupies a
contiguous block of columns holding 128*q fixed-length segment slots.

The device kernel is then a pure batched row-block log-softmax with static
shapes: exp (ScalarE) -> per-slot reduce_sum (VectorE) -> ln (ScalarE) ->
broadcast subtract (VectorE), streamed in ~2k-column groups overlapped with
HBM loads/stores. out = x - log(sum(exp(x))) is mathematically identical to
the reference's max-normalized form, and with standard-normal logits fp32
exp/log are nowhere near overflow, so skipping the max pass is numerically
safe (measured absmax error ~1e-5 against the fp32 reference).

Length-1 segments are exactly 0 in the reference, so they are filled on the
host. Empty segments produce no output elements.
"""
from contextlib import ExitStack

import numpy as np

N_CORES = 8
P = 128
PAD_VAL = -80.0

# canonical slot lengths: exact for 2..24, coarser for the rare tail
_CANON_BASE = list(range(2, 25)) + [26, 28, 30, 32, 36, 40, 44, 48, 56, 64, 80, 96, 128]


def _canon_lengths(max_len):
    canon = list(_CANON_BASE)
    while canon[-1] < max_len:
        canon.append(canon[-1] * 2)
    return np.asarray(canon, dtype=np.int64)


def _plan_buckets(index, num_segments):
    """Placement plan: maps every element to (core, flat offset) in the padded
    per-core [128, W_total] layout."""
    S = int(num_segments)
    idx = np.asarray(index).astype(np.int64)
    L = np.bincount(idx, minlength=S)
    starts = np.zeros(S + 1, dtype=np.int64)
    np.cumsum(L, out=starts[1:])

    seg1 = np.where(L == 1)[0]
    sel = np.where(L >= 2)[0]
    plan = dict(seg1=seg1, starts=starts)
    if len(sel) == 0:
        plan.update(W_total=0, buckets=[], e_src=np.empty(0, np.int64),
                    e_coreflat=np.empty(0, np.int64))
        return plan
    Ls = L[sel]
    canon = _canon_lengths(int(Ls.max()))
    Lc = canon[np.searchsorted(canon, Ls, side="left")]

    order = np.argsort(Lc, kind="stable")
    segs_sorted = sel[order]
    Ls_sorted = Ls[order]
    Lc_sorted = Lc[order]

    uniq, ustart, ucount = np.unique(Lc_sorted, return_index=True, return_counts=True)

    buckets = []                               # (Lb, q_b, col_b)
    col = 0
    nseg = len(segs_sorted)
    seg_core = np.empty(nseg, dtype=np.int64)
    seg_col = np.empty(nseg, dtype=np.int64)
    seg_prow = np.empty(nseg, dtype=np.int64)
    for Lb, s0, n in zip(uniq, ustart, ucount):
        Lb = int(Lb); s0 = int(s0); n = int(n)
        c = -(-n // N_CORES)                   # segs per core (ceil)
        q = -(-c // P)                         # slots per partition
        j = np.arange(n)
        core = j // c
        j_loc = j - core * c
        p = j_loc // q
        t = j_loc - p * q
        seg_core[s0:s0 + n] = core
        seg_prow[s0:s0 + n] = p
        seg_col[s0:s0 + n] = col + t * Lb
        buckets.append((Lb, q, col))
        col += q * Lb
    W_total = col

    tot_el = int(Ls_sorted.sum())
    off = np.zeros(nseg + 1, dtype=np.int64)
    np.cumsum(Ls_sorted, out=off[1:])
    within = np.arange(tot_el) - np.repeat(off[:-1], Ls_sorted)
    e_src = np.repeat(starts[segs_sorted], Ls_sorted) + within
    flat = seg_prow * W_total + seg_col
    e_flat = np.repeat(flat, Ls_sorted) + within
    e_core = np.repeat(seg_core, Ls_sorted)
    plan.update(W_total=W_total, buckets=buckets, e_src=e_src,
                e_coreflat=e_core * (P * W_total) + e_flat)
    return plan


def _build_inputs(logits, plan):
    W_total = plan["W_total"]
    xin = np.full(N_CORES * P * W_total, PAD_VAL, dtype=np.float32)
    xin[plan["e_coreflat"]] = np.asarray(logits, dtype=np.float32)[plan["e_src"]]
    return xin.reshape(N_CORES, P * W_total)


def _gather_output(results_flat, plan, n):
    out = np.zeros(n, dtype=np.float32)
    out[plan["e_src"]] = results_flat.reshape(-1)[plan["e_coreflat"]]
    out[plan["starts"][plan["seg1"]]] = 0.0
    return out


def _make_groups(buckets, target=2048, cap=2560):
    """Split bucket column ranges into contiguous ~target-column groups of
    whole segment slots; each group is a list of (col, q_slice, Lb)."""
    slices = []
    for (Lb, q, col) in buckets:
        qk = max(1, target // Lb)
        t = 0
        while t < q:
            qs = min(qk, q - t)
            slices.append((col + t * Lb, qs, Lb))
            t += qs
    groups, cur, cur_cols = [], [], 0
    for s in slices:
        scols = s[1] * s[2]
        if cur and cur_cols + scols > cap:
            groups.append(cur)
            cur, cur_cols = [], 0
        cur.append(s)
        cur_cols += scols
    if cur:
        groups.append(cur)
    return groups


def _build_program(W_total, buckets, ebufs=3, target=2048, cap=2560, n_stages=2):
    """Two-stage pipeline (best measured): stage B's loads/exp/reduce overlap
    stage A's subtract/store. Loads issue on the sync HWDGE ring, stores on the
    scalar HWDGE ring (no FIFO head-of-line blocking between them). Per-stage
    Ln keeps ACT table switches to 4 total. x tiles persist per group; the
    subtract runs in place on x."""
    import concourse.bacc as bacc
    import concourse.mybir as mybir
    from concourse import tile

    F32 = mybir.dt.float32
    nc = bacc.Bacc("TRN2", target_bir_lowering=False, debug=False,
                   num_devices=N_CORES)
    xin = nc.dram_tensor("xin", [P * W_total], F32, kind="ExternalInput").ap()
    xout = nc.dram_tensor("xout", [P * W_total], F32, kind="ExternalOutput").ap()
    xin2d = xin.rearrange("(p w) -> p w", p=P)
    xout2d = xout.rearrange("(p w) -> p w", p=P)

    groups = _make_groups(buckets, target=target, cap=cap)
    Q_total = sum(qs for g in groups for (_, qs, _) in g)

    # split groups into n_stages consecutive chunks, balanced by columns
    gcols = [g[-1][0] + g[-1][1] * g[-1][2] - g[0][0] for g in groups]
    total_cols = sum(gcols)
    stages, cur, acc = [], [], 0
    for g, gc in zip(groups, gcols):
        cur.append(g)
        acc += gc
        if (acc >= total_cols * (len(stages) + 1) / n_stages - 1
                and len(stages) < n_stages - 1):
            stages.append(cur)
            cur = []
    if cur:
        stages.append(cur)

    qof, xts = {}, {}

    with tile.TileContext(nc) as tc, ExitStack() as ctx:
        xpool = ctx.enter_context(tc.tile_pool(name="x", bufs=1))
        epool = ctx.enter_context(tc.tile_pool(name="e", bufs=ebufs))
        spool = ctx.enter_context(tc.tile_pool(name="s", bufs=1))

        st = spool.tile([P, Q_total], F32, tag="s")
        ct = spool.tile([P, Q_total], F32, tag="c")
        qoff = 0
        gid = 0

        def phaseA(g):
            nonlocal qoff, gid
            g0, g1 = g[0][0], g[-1][0] + g[-1][1] * g[-1][2]
            xt = xpool.tile([P, g1 - g0], F32, tag=f"x{gid}")
            xts[gid] = xt
            nc.sync.dma_start(xt[:], xin2d[:, g0:g1])
            et = epool.tile([P, g1 - g0], F32, tag="e")
            nc.scalar.activation(et[:], xt[:], mybir.ActivationFunctionType.Exp)
            qof[gid] = qoff
            for (col, qs, Lb) in g:
                c0 = col - g0
                nc.vector.reduce_sum(
                    st[:, qoff:qoff + qs],
                    et[:, c0:c0 + qs * Lb].rearrange("p (q l) -> p q l", q=qs),
                    axis=mybir.AxisListType.X)
                qoff += qs
            gid += 1

        def phaseC(g, i):
            g0, g1 = g[0][0], g[-1][0] + g[-1][1] * g[-1][2]
            xt = xts[i]
            q = qof[i]
            for (col, qs, Lb) in g:
                c0 = col - g0
                nc.vector.tensor_sub(
                    xt[:, c0:c0 + qs * Lb].rearrange("p (q l) -> p q l", q=qs),
                    xt[:, c0:c0 + qs * Lb].rearrange("p (q l) -> p q l", q=qs),
                    ct[:, q:q + qs].unsqueeze(2).broadcast_to([P, qs, Lb]))
                q += qs
            nc.scalar.dma_start(xout2d[:, g0:g1], xt[:])

        stage_ids = []
        for si, stage in enumerate(stages):
            q0 = qoff
            ids = []
            for g in stage:
                ids.append((g, gid))
                phaseA(g)
            stage_ids.append(ids)
            nc.scalar.activation(ct[:, q0:qoff], st[:, q0:qoff],
                                 mybir.ActivationFunctionType.Ln)
            if si > 0:
                # subtract/store of the previous stage overlaps this stage's
                # compute tail and the loads already in flight
                for (g, i) in stage_ids[si - 1]:
                    phaseC(g, i)
        for (g, i) in stage_ids[-1]:
            phaseC(g, i)
    nc.compile()
    return nc


_cache = {}


def _get_program(plan):
    key = (plan["W_total"], tuple(plan["buckets"]))
    if key not in _cache:
        _cache[key] = _build_program(plan["W_total"], plan["buckets"])
    return _cache[key]


def run_on_device(nc, xin_cores, trace=False, **kw):
    from concourse.bass_utils import run_bass_kernel_spmd
    in_maps = [{"xin": xin_cores[c]} for c in range(N_CORES)]
    res = run_bass_kernel_spmd(nc, in_maps, core_ids=list(range(N_CORES)),
                               trace=trace, **kw)
    out = np.stack([res.results[c]["xout"] for c in range(N_CORES)])
    return out, res


def kernel(logits, index, num_segments):
    logits = np.asarray(logits)
    n = logits.shape[0]
    plan = _plan_buckets(index, num_segments)
    if plan["W_total"] == 0:
        out = np.zeros(n, dtype=np.float32)
        out[plan["starts"][plan["seg1"]]] = 0.0
        return out
    xin = _build_inputs(logits, plan)
    nc = _get_program(plan)
    out_flat, _ = run_on_device(nc, xin)
    return _gather_output(out_flat, plan, n)



# revision 2
# speedup vs baseline: 1.1610x; 1.1610x over previous
"""Grouped categorical log-softmax (segment logsumexp) on 8 Trainium2 cores.

v2 design (from perfetto/ntff analysis of the 65us baseline):
  * fp16 device I/O halves HBM traffic (error ~1e-3 << 2e-2 gate).
  * Fine-grained software pipeline (~1k-folded-col chunks, 4-deep): loads,
    exp, fold+reduce, ln, subtract and stores of neighbouring chunks all
    overlap, so the load and store DMA streams run concurrently (the
    baseline serialized them: loads 8-35us, stores 34-62us).
  * One ACT table load: a manual InstLoadActFuncSet of the
    natural_log_exp_and_others set replaces the 4 Exp<->Ln switches
    (~2.7us each) the compiler's first-fit placement produced.
  * Segment slots are padded to even canonical lengths and each slot's
    elements are split across the two halves of its chunk, so one fp16
    tensor_add at 2x ("fold") halves the 1x reduce_sum work.
  * The Ln activation writes its result pre-broadcast per slot (dense
    fp16 output), so the subtract is a single whole-chunk dense fp16
    tensor_tensor at 2x instead of a 1x broadcast subtract.

Per-core steady state: DVE ~3.2us/chunk paces; ACT (exp+ln) just under;
DMA fp16 well under.  Padding -12 (not -inf/-80) keeps exp() results
subnormal-positive in fp16 so empty/padded slots give finite ln.

Length-1 segments are exactly 0 in the reference and are filled on the
host.  Empty segments produce no output elements.
"""
from contextlib import ExitStack

import numpy as np

N_CORES = 8
P = 128
PAD_VAL = -12.0          # exp(-12) ~ 6e-6: fp16-subnormal, >0 so ln stays finite
CHUNK_TARGET = 1024      # folded columns per chunk (tunable)
ACT_SET_NL_EXP = 6       # natural_log_exp_and_others in act_info.json


def _canon_lengths(max_len):
    canon = list(range(2, 49, 2)) + [56, 64, 80, 96, 128]
    while canon[-1] < max_len:
        canon.append(canon[-1] * 2)
    return np.asarray(canon, dtype=np.int64)


def _plan_buckets(index, num_segments):
    """Placement plan mapping every element to (core, flat offset) in the
    per-core [128, W] fp16 layout, plus the chunk/region metadata the device
    program is built from."""
    S = int(num_segments)
    idx = np.asarray(index).astype(np.int64)
    L = np.bincount(idx, minlength=S)
    starts = np.zeros(S + 1, dtype=np.int64)
    np.cumsum(L, out=starts[1:])

    seg1 = np.where(L == 1)[0]
    sel = np.where(L >= 2)[0]
    plan = dict(seg1=seg1, starts=starts)
    if len(sel) == 0:
        plan.update(W=0, chunks=(), Q_total=0,
                    e_src=np.empty(0, np.int64), e_coreflat=np.empty(0, np.int64))
        return plan
    Ls = L[sel]
    canon = _canon_lengths(int(Ls.max()))
    Lc = canon[np.searchsorted(canon, Ls, side="left")]

    order = np.argsort(Lc, kind="stable")
    segs_sorted = sel[order]
    Ls_sorted = Ls[order]
    Lc_sorted = Lc[order]
    uniq, ustart, ucount = np.unique(Lc_sorted, return_index=True, return_counts=True)

    # --- chunk construction (folded coordinate space) ------------------
    # chunk = dict(Ghc, regions=[(rel_off, nslots, H, qoff)]); runs remember
    # which chunk each slot-range of each bucket landed in.
    chunks = []
    cur_regions, cur_ghc = [], 0
    qoff = 0
    bucket_runs = {}  # Lb -> (t0s, chunk_ids, rel_offs) arrays

    def close_chunk():
        nonlocal cur_regions, cur_ghc
        if cur_regions:
            chunks.append(dict(Ghc=cur_ghc, regions=tuple(cur_regions)))
            cur_regions, cur_ghc = [], 0

    binfo = []  # (Lb, s0, n, c, q, H)
    for Lb, s0, n in zip(uniq, ustart, ucount):
        Lb = int(Lb); s0 = int(s0); n = int(n)
        c = -(-n // N_CORES)
        q = -(-c // P)
        H = Lb // 2
        binfo.append((Lb, s0, n, c, q, H))
        t0s, cids, rels = [], [], []
        t = 0
        while t < q:
            k = (CHUNK_TARGET - cur_ghc) // H
            if k <= 0:
                close_chunk()
                k = max(1, CHUNK_TARGET // H)
            k = min(k, q - t)
            t0s.append(t); cids.append(len(chunks)); rels.append(cur_ghc)
            cur_regions.append((cur_ghc, k, H, qoff))
            qoff += k
            cur_ghc += k * H
            t += k
            if cur_ghc >= CHUNK_TARGET:
                close_chunk()
        bucket_runs[Lb] = (np.array(t0s + [q]), np.array(cids), np.array(rels))
    close_chunk()
    Q_total = qoff

    ghcs = np.array([ch["Ghc"] for ch in chunks], dtype=np.int64)
    bases = np.zeros(len(chunks) + 1, dtype=np.int64)
    np.cumsum(2 * ghcs, out=bases[1:])
    W = int(bases[-1])
    for ch, b in zip(chunks, bases[:-1]):
        ch["base"] = int(b)

    # --- per-segment placement ----------------------------------------
    nseg = len(segs_sorted)
    seg_core = np.empty(nseg, dtype=np.int64)
    seg_prow = np.empty(nseg, dtype=np.int64)
    seg_col0 = np.empty(nseg, dtype=np.int64)   # full-col base of slot (left half)
    seg_ghc = np.empty(nseg, dtype=np.int64)    # chunk folded width
    seg_H = np.empty(nseg, dtype=np.int64)
    for Lb, s0, n, c, q, H in binfo:
        j = np.arange(n)
        core = j // c
        j_loc = j - core * c
        p = j_loc // q
        t = j_loc - p * q
        t0s, cids, rels = bucket_runs[Lb]
        r = np.searchsorted(t0s, t, side="right") - 1
        ch_id = cids[r]
        rel = rels[r] + (t - t0s[r]) * H
        sl = slice(s0, s0 + n)
        seg_core[sl] = core
        seg_prow[sl] = p
        seg_col0[sl] = bases[ch_id] + rel
        seg_ghc[sl] = ghcs[ch_id]
        seg_H[sl] = H

    tot_el = int(Ls_sorted.sum())
    off = np.zeros(nseg + 1, dtype=np.int64)
    np.cumsum(Ls_sorted, out=off[1:])
    within = np.arange(tot_el) - np.repeat(off[:-1], Ls_sorted)
    e_src = np.repeat(starts[segs_sorted], Ls_sorted) + within
    rH = np.repeat(seg_H, Ls_sorted)
    e_col = (np.repeat(seg_col0, Ls_sorted) + within
             + (within >= rH) * (np.repeat(seg_ghc, Ls_sorted) - rH))
    e_flat = np.repeat(seg_prow, Ls_sorted) * W + e_col
    e_core = np.repeat(seg_core, Ls_sorted)
    plan.update(W=W, Q_total=Q_total, e_src=e_src,
                e_coreflat=e_core * (P * W) + e_flat,
                chunks=tuple((ch["base"], ch["Ghc"], ch["regions"])
                             for ch in chunks))
    return plan


def _build_inputs(logits, plan):
    W = plan["W"]
    xin = np.full(N_CORES * P * W, PAD_VAL, dtype=np.float16)
    xin[plan["e_coreflat"]] = np.asarray(logits, dtype=np.float16)[plan["e_src"]]
    return xin.reshape(N_CORES, P * W)


def _gather_output(results_flat, plan, n):
    out = np.zeros(n, dtype=np.float32)
    out[plan["e_src"]] = results_flat.reshape(-1)[plan["e_coreflat"]].astype(np.float32)
    out[plan["starts"][plan["seg1"]]] = 0.0
    return out


def _build_program(W, chunks, Q_total):
    import concourse.bacc as bacc
    import concourse.mybir as mybir
    from concourse import tile

    F16 = mybir.dt.float16
    F32 = mybir.dt.float32
    Exp = mybir.ActivationFunctionType.Exp
    Ln = mybir.ActivationFunctionType.Ln
    AX = mybir.AxisListType.X

    nc = bacc.Bacc("TRN2", target_bir_lowering=False, debug=False,
                   num_devices=N_CORES)
    xin = nc.dram_tensor("xin", [P * W], F16, kind="ExternalInput").ap()
    xout = nc.dram_tensor("xout", [P * W], F16, kind="ExternalOutput").ap()
    xin2d = xin.rearrange("(p w) -> p w", p=P)
    xout2d = xout.rearrange("(p w) -> p w", p=P)

    # One table set containing both exp and ln, loaded up front; the
    # compiler's own pass then sees every activation covered on all paths.
    nc.scalar.add_instruction(mybir.InstLoadActFuncSet(
        name=nc.scalar.bass.get_next_instruction_name(), ins=[], outs=[],
        act_func_set_id=ACT_SET_NL_EXP))

    n = len(chunks)
    gcap = 2 * max(ch[1] for ch in chunks)

    with tile.TileContext(nc) as tc, ExitStack() as ctx:
        xp = ctx.enter_context(tc.tile_pool(name="x", bufs=6))
        ep = ctx.enter_context(tc.tile_pool(name="e", bufs=6))
        fp = ctx.enter_context(tc.tile_pool(name="f", bufs=3))
        cp = ctx.enter_context(tc.tile_pool(name="c", bufs=3))
        sp = ctx.enter_context(tc.tile_pool(name="s", bufs=1))
        st = sp.tile([P, Q_total], F32, tag="st")
        X, E, C = {}, {}, {}

        with nc.allow_low_precision("fp16 data path by design"):
            for g in range(n + 4):
                if g < n:
                    base, ghc, _ = chunks[g]
                    xt = xp.tile([P, gcap], F16, tag="x")
                    X[g] = xt
                    nc.sync.dma_start(xt[:, :2 * ghc], xin2d[:, base:base + 2 * ghc])
                if 0 <= g - 1 < n:
                    i = g - 1
                    _, ghc, _ = chunks[i]
                    et = ep.tile([P, gcap], F16, tag="e")
                    E[i] = et
                    nc.scalar.activation(et[:, :2 * ghc], X[i][:, :2 * ghc], Exp)
                if 0 <= g - 2 < n:
                    i = g - 2
                    _, ghc, regions = chunks[i]
                    ft = fp.tile([P, gcap // 2], F16, tag="f")
                    et = E[i]
                    nc.vector.tensor_add(ft[:, :ghc], et[:, :ghc], et[:, ghc:2 * ghc])
                    for (rel, q, H, qo) in regions:
                        nc.vector.reduce_sum(
                            st[:, qo:qo + q],
                            ft[:, rel:rel + q * H].rearrange("p (q h) -> p q h", q=q),
                            axis=AX)
                if 0 <= g - 3 < n:
                    i = g - 3
                    _, ghc, regions = chunks[i]
                    ct = cp.tile([P, gcap // 2], F16, tag="c")
                    C[i] = ct
                    for (rel, q, H, qo) in regions:
                        nc.scalar.activation(
                            ct[:, rel:rel + q * H].rearrange("p (q h) -> p q h", q=q),
                            st[:, qo:qo + q].unsqueeze(2).broadcast_to([P, q, H]),
                            Ln)
                if 0 <= g - 4 < n:
                    i = g - 4
                    base, ghc, _ = chunks[i]
                    xt, et, ct = X.pop(i), E.pop(i), C.pop(i)
                    nc.vector.tensor_sub(
                        et[:, :2 * ghc].rearrange("p (s h) -> p s h", s=2),
                        xt[:, :2 * ghc].rearrange("p (s h) -> p s h", s=2),
                        ct[:, :ghc].unsqueeze(1).broadcast_to([P, 2, ghc]))
                    nc.scalar.dma_start(xout2d[:, base:base + 2 * ghc],
                                        et[:, :2 * ghc])
    nc.compile()
    return nc


_cache = {}


def _get_program(plan):
    key = (plan["W"], plan["Q_total"], plan["chunks"])
    if key not in _cache:
        _cache[key] = _build_program(plan["W"], plan["chunks"], plan["Q_total"])
    return _cache[key]


def run_on_device(nc, xin_cores, trace=False, **kw):
    from concourse.bass_utils import run_bass_kernel_spmd
    in_maps = [{"xin": xin_cores[c]} for c in range(N_CORES)]
    res = run_bass_kernel_spmd(nc, in_maps, core_ids=list(range(N_CORES)),
                               trace=trace, **kw)
    out = np.stack([res.results[c]["xout"] for c in range(N_CORES)])
    return out, res


def kernel(logits, index, num_segments):
    logits = np.asarray(logits)
    n = logits.shape[0]
    plan = _plan_buckets(index, num_segments)
    if plan["W"] == 0:
        out = np.zeros(n, dtype=np.float32)
        out[plan["starts"][plan["seg1"]]] = 0.0
        return out
    xin = _build_inputs(logits, plan)
    nc = _get_program(plan)
    out_flat, _ = run_on_device(nc, xin)
    return _gather_output(out_flat, plan, n)
